# revision 19
# baseline (speedup 1.0000x reference)
"""Trainium2 Bass kernel for fused multi-head attention (16 heads, d=64,
b=2, n=2048, h=1024) across 8 NeuronCores.

Sharding: tensor-parallel over heads x data-parallel over batch.
Core c handles batch c//4 and heads [4*(c%4), 4*(c%4)+4). Each core
computes Q^T/K^T/V for its 4 heads over the full 2048-token sequence from
a replicated (per-batch) x — no communication before attention, so the
TensorE/ScalarE pipeline starts immediately and the kernel-start
cross-core barrier is hidden. After attention, a small bf16 AllToAll per
512-row piece (Ulysses-style) swaps head-shards for row-shards, so each
core runs the output projection locally with the full 1024 contraction —
no reduction collective and only ~1MB of total wire traffic per core.

Attention per head runs in scores-transposed layout [k, q] (softmax
without max subtraction -- logits are ~N(0,1) here), the softmax
denominator comes from a ones-column appended to V (M=65 AV matmuls), and
normalization is applied to the small attn_out^T [64, 512] tiles via a PE
broadcast of 1/denom. The attn^T layout feeds both the AllToAll and the
output projection lhsT directly, so no on-chip transposes are needed
anywhere. All matmuls bf16 with f32 PSUM accumulation; softmax exp on
ScalarE is the pace-setting engine (~147 us/core).
"""

import sys

if "/opt/trn_rl_repo" not in sys.path:
    sys.path.insert(0, "/opt/trn_rl_repo")

import numpy as np
import ml_dtypes

import concourse.bass as bass
import concourse.mybir as mybir
import concourse.tile as tile
from concourse import bacc
from concourse.bass import ts
from concourse.bass_utils import run_bass_kernel_spmd

BF16 = mybir.dt.bfloat16
F32 = mybir.dt.float32
ADD = mybir.AluOpType.add
MULT = mybir.AluOpType.mult
BYPASS = mybir.AluOpType.bypass
EXP = mybir.ActivationFunctionType.Exp

HEADS, D, H, N, B = 16, 64, 1024, 2048, 2
NC_ = 8
LH = 4            # local heads per core
LPAIRS = 2        # local head pairs
KC = 16           # k chunks of 128 over n=2048
QC = 4            # q chunks of 512 over n=2048 (= AllToAll pieces)
LVW = LH * 65     # 260: local v-aug width
LQK = LH * D      # 256 local q (or k) columns
GROUPS = [[0, 1, 2, 3], [4, 5, 6, 7]]


def build_nc():
    nc = bacc.Bacc("TRN2", target_bir_lowering=False, debug=False, num_devices=NC_)

    xT = nc.declare_dram_parameter("xT", [H, N], BF16, isOutput=False)
    wqk = nc.declare_dram_parameter("wqk", [H, 2 * LQK], BF16, isOutput=False)
    wv = nc.declare_dram_parameter("wv", [H, LVW], BF16, isOutput=False)
    wout = nc.declare_dram_parameter("wout", [H, H], BF16, isOutput=False)
    cos2 = nc.declare_dram_parameter("cos2", [128, N], F32, isOutput=False)
    # sswp[p] = sin value read at SOURCE partition p during the shuffle:
    # p%64 < 32 -> +sin[p%64+32], else -sin[p%64-32]
    sinm = nc.declare_dram_parameter("sinm", [128, N], F32, isOutput=False)
    # msk[:,0]=1 iff this core's batch is 0; msk[:,1]=1 iff batch 1
    msk = nc.declare_dram_parameter("msk", [128, 2], F32, isOutput=False)
    out = nc.declare_dram_parameter("out", [QC, 128, H], F32, isOutput=True)

    with tile.TileContext(nc) as tc:
        with (
            tc.tile_pool(name="dram", bufs=1, space="DRAM") as dram,
            tc.tile_pool(name="sb", bufs=1) as sb,
            tc.tile_pool(name="sbw", bufs=1) as sbw,
            tc.tile_pool(name="psum", bufs=2, space="PSUM") as ps,
        ):
            a2a_in = [dram.tile([8, 2 * 128, 128], BF16, name=f"ain{i}")
                      for i in range(QC)]
            a2a_out = [dram.tile([8, 2 * 128, 128], BF16, name=f"aout{i}")
                       for i in range(QC)]

            # ---- stage inputs; x split across three DMA queues ----
            xt_sb = sbw.tile([128, 8 * N], BF16)
            wqk_sb = sbw.tile([128, 8 * 2 * LQK], BF16)
            wv_sb = sbw.tile([128, 8 * LVW], BF16)
            wout_sb = sbw.tile([128, 8 * H], BF16)
            cos2_sb = sbw.tile([128, N], F32)
            sinm_sb = sbw.tile([128, N], F32)
            ones_sb = sbw.tile([1, D], F32)
            msk_sb = sbw.tile([128, 2], F32)
            nc.sync.dma_start(msk_sb[:, :], msk[:, :])
            for hk in range(8):
                nc.sync.dma_start(wqk_sb[:, ts(hk, 2 * LQK)], wqk[ts(hk, 128), :])
            engs = [nc.sync, nc.gpsimd, nc.scalar]
            for hk in range(8):
                engs[hk % 3].dma_start(xt_sb[:, ts(hk, N)], xT[ts(hk, 128), :])
            nc.gpsimd.dma_start(cos2_sb[:, :], cos2[:, :])
            nc.gpsimd.dma_start(sinm_sb[:, :], sinm[:, :])
            for hk in range(8):
                nc.sync.dma_start(wv_sb[:, ts(hk, LVW)], wv[ts(hk, 128), :])
            for hk in range(8):
                nc.sync.dma_start(wout_sb[:, ts(hk, H)], wout[ts(hk, 128), :])
            nc.vector.memset(ones_sb[:, :], 1.0)

            kt_rot = sb.tile([128, 2 * N], BF16)   # [pair pr at pr*N][n]
            qt_rot = sb.tile([128, 2 * N], BF16)
            vt_all = sb.tile([128, KC * LVW], BF16)
            attn_sb = sb.tile([128, 2 * N], BF16)  # attn^T [pair][qc*512+row]

            def rotary(p, dst_ap, sc):
                """dst = p*cos + shuffle32(p)*sin for n-slice sc (512 wide)."""
                tmp = sb.tile([128, 512], F32, tag="rota", bufs=2, name="rota")
                tmp2 = sb.tile([128, 512], F32, tag="rotb", bufs=2, name="rotb")
                cs = cos2_sb[:, ts(sc, 512)]
                sn = sinm_sb[:, ts(sc, 512)]
                for hh in (0, 64):
                    nc.vector.tensor_tensor(
                        tmp[hh : hh + 32, :], p[hh + 32 : hh + 64, :],
                        sn[hh + 32 : hh + 64, :], MULT)
                    nc.vector.tensor_tensor(
                        tmp[hh + 32 : hh + 64, :], p[hh : hh + 32, :],
                        sn[hh : hh + 32, :], MULT)
                nc.vector.tensor_tensor(tmp2[:, :], p[:, :], cs[:, :], MULT)
                nc.vector.tensor_tensor(dst_ap, tmp2[:, :], tmp[:, :], ADD)

            def proj_group(col0, pr, sc):
                p = ps.tile([128, 512], F32, tag="b", name="pp")
                for hk in range(8):
                    nc.tensor.matmul(
                        p[:, :],
                        lhsT=wqk_sb[:, hk * 2 * LQK + col0 + pr * 128:][:, :128],
                        rhs=xt_sb[:, hk * N + sc * 512:][:, :512],
                        start=(hk == 0),
                        stop=(hk == 7),
                    )
                return p

            def q_proj_rot(pr, sc):
                p = proj_group(0, pr, sc)
                rotary(p, qt_rot[:, pr * N + sc * 512:][:, :512], sc)

            # K pair0 first (DVE rotary straight from PSUM): gates attention
            for sc in range(4):
                p = proj_group(LQK, 0, sc)
                rotary(p, kt_rot[:, sc * 512:][:, :512], sc)
            q_proj_rot(0, 0)

            # V projection straight into per-kc tiles (+ ones columns)
            for rc in range(KC):
                p = ps.tile([128, LVW], F32, tag="b", name="vp")
                for hk in range(8):
                    nc.tensor.matmul(
                        p[:, :],
                        lhsT=xt_sb[:, hk * N + rc * 128:][:, :128],
                        rhs=wv_sb[:, ts(hk, LVW)],
                        start=(hk == 0),
                        stop=(hk == 7),
                    )
                nc.scalar.copy(vt_all[:, ts(rc, LVW)], p[:, :])
                ones_view = vt_all[:, ts(rc, LVW)].rearrange(
                    "p (h e) -> p h e", e=65)[:, :, 64:65]
                nc.vector.memset(ones_view, 1.0)

            # ---- attention; per-piece AllToAll + local output projection ----
            def emit_norm(st):
                av0, av1, qc, pr = st
                rc0 = sb.tile([1, 512], F32, tag="rcp", bufs=4, name="rc0")
                rc1 = sb.tile([1, 512], F32, tag="rcp", bufs=4, name="rc1")
                rd0 = sb.tile([1, 512], F32, tag="rcd", bufs=4, name="rd0")
                rd1 = sb.tile([1, 512], F32, tag="rcd", bufs=4, name="rd1")
                nc.vector.tensor_copy(rd0[:, :], av0[64:65, :])
                nc.vector.tensor_copy(rd1[:, :], av1[64:65, :])
                nc.vector.reciprocal_approx_fast(out=rc0[:, :], in_=rd0[:, :])
                nc.vector.reciprocal_approx_fast(out=rc1[:, :], in_=rd1[:, :])
                b_ps = ps.tile([128, 512], F32, tag="b", name="b_ps")
                nc.tensor.matmul(b_ps[0:64, :], lhsT=ones_sb[:, :],
                                 rhs=rc0[:, :], start=True, stop=True,
                                 tile_position=(0, 0))
                nc.tensor.matmul(b_ps[64:128, :], lhsT=ones_sb[:, :],
                                 rhs=rc1[:, :], start=True, stop=True,
                                 tile_position=(0, 64))
                b_sb = sb.tile([128, 512], F32, tag="bsb", bufs=2, name="b_sb")
                nc.vector.tensor_copy(b_sb[:, :], b_ps[:, :])
                dst = attn_sb[:, pr * N + qc * 512:][:, :512]
                nc.vector.tensor_tensor(dst[0:64, :], av0[0:64, :],
                                        b_sb[0:64, :], MULT)
                nc.vector.tensor_tensor(dst[64:128, :], av1[0:64, :],
                                        b_sb[64:128, :], MULT)

            def emit_a2a(qc):
                # shard j of a2a_in = my 2 head-pair chunks for row block
                # j%4, duplicated to both batch groups (receiver masks off
                # the cross-batch half)
                src3 = attn_sb.rearrange("p (pr x) -> p pr x", pr=2)
                for j in range(8):
                    nc.sync.dma_start(
                        a2a_in[qc][j].rearrange("(pr p) x -> p pr x", p=128),
                        src3[:, :, qc * 512 + 128 * (j % 4):][:, :, :128])
                nc.gpsimd.collective_compute(
                    "AllToAll", BYPASS, replica_groups=[list(range(8))],
                    ins=[a2a_in[qc].opt()], outs=[a2a_out[qc].opt()])

            def emit_outproj(qc):
                # raw slots from all 8 ranks, then mask-combine batch halves
                att_r = sb.tile([128, 16 * 128], BF16, tag="attr", bufs=2,
                                name="att_r")
                r3 = att_r.rearrange("p (c x) -> p c x", x=128)
                for i in range(8):
                    nc.sync.dma_start(
                        r3[:, 2 * i : 2 * i + 2, :],
                        a2a_out[qc][i].rearrange("(c p) x -> p c x", p=128))
                att_g = sb.tile([128, 8 * 128], BF16, tag="attg", bufs=2,
                                name="att_g")
                g3 = att_g.rearrange("p (c x) -> p c x", x=128)
                tmpm = sb.tile([128, 128], BF16, tag="tmpm", bufs=2, name="tmpm")
                for hc in range(8):
                    lo = r3[:, (hc // 2) * 2 + (hc % 2), :]
                    hi = r3[:, 8 + (hc // 2) * 2 + (hc % 2), :]
                    nc.vector.tensor_scalar_mul(tmpm[:, :], hi, msk_sb[:, 1:2])
                    nc.vector.scalar_tensor_tensor(
                        g3[:, hc, :], lo, msk_sb[:, 0:1], tmpm[:, :],
                        MULT, ADD)
                for nh in range(2):
                    o_ps = ps.tile([128, 512], F32, tag="b", name="o_ps")
                    for hc in range(8):
                        nc.tensor.matmul(
                            o_ps[:, :],
                            lhsT=g3[:, hc, :],
                            rhs=wout_sb[:, hc * H + nh * 512:][:, :512],
                            start=(hc == 0),
                            stop=(hc == 7),
                        )
                    ob = sb.tile([128, 512], F32, tag="ob", bufs=3, name="ob")
                    nc.vector.tensor_copy(ob[:, :], o_ps[:, :])
                    nc.sync.dma_start(out[qc, :, ts(nh, 512)], ob[:, :])

            norm_pending = None   # (av0, av1, qc, pr)
            a2a_ready = []        # pieces normalized, awaiting A2A emission
            op_ready = []         # pieces with A2A emitted, awaiting outproj
            for qc in range(QC):
                if qc + 1 < QC:  # prefetch next piece's Q rotary
                    q_proj_rot(0, qc + 1)
                    q_proj_rot(1, qc + 1)
                for pr in range(LPAIRS):
                    qt_p = qt_rot[:, pr * N + qc * 512:][:, :512]
                    av0 = ps.tile([65, 512], F32, tag="av", name="av0")
                    av1 = ps.tile([65, 512], F32, tag="av", name="av1")
                    exps = []
                    for kc in range(KC):
                        s_ps = ps.tile([128, 1024], F32, tag="s", name="s_ps")
                        nc.tensor.matmul(
                            s_ps[:, 0:512],
                            lhsT=kt_rot[0:64, pr * N + kc * 128:][:, :128],
                            rhs=qt_p[0:64, :], start=True, stop=True,
                            tile_position=(0, 0))
                        nc.tensor.matmul(
                            s_ps[:, 512:1024],
                            lhsT=kt_rot[64:128, pr * N + kc * 128:][:, :128],
                            rhs=qt_p[64:128, :], start=True, stop=True,
                            tile_position=(64, 0))
                        e = sb.tile([128, 1024], BF16, tag="exp", bufs=4, name="e")
                        nc.scalar.activation(e[:, :], s_ps[:, :], EXP, scale=0.125)
                        exps.append(e)
                        if qc == 0 and pr == 0 and kc in (2, 4, 6, 8):
                            sc = (kc - 2) // 2
                            p2 = proj_group(LQK, 1, sc)
                            rotary(p2, kt_rot[:, N + sc * 512:][:, :512], sc)
                        if qc == 0 and pr == 0 and kc == 10:
                            q_proj_rot(1, 0)
                        if kc == 1 and norm_pending is not None:
                            emit_norm(norm_pending)
                            if norm_pending[3] == 1:  # piece complete
                                a2a_ready.append(norm_pending[2])
                            norm_pending = None
                        if kc == 3 and a2a_ready:
                            emit_a2a(a2a_ready.pop(0))
                            op_ready.append(qc - 1)
                        if kc == 8 and pr == 1 and op_ready:
                            emit_outproj(op_ready.pop(0))
                        if kc > 0:
                            _av_mm(nc, vt_all, exps[kc - 1], av0, av1, kc - 1, pr)
                    _av_mm(nc, vt_all, exps[KC - 1], av0, av1, KC - 1, pr)
                    norm_pending = (av0, av1, qc, pr)
            emit_norm(norm_pending)
            emit_a2a(QC - 1)
            for p_ in op_ready:
                emit_outproj(p_)
            emit_outproj(QC - 1)

    nc.finalize()
    return nc


def _av_mm(nc, vt_all, e, av0, av1, kc, pr):
    nc.tensor.matmul(
        av0[:, :], lhsT=vt_all[:, kc * LVW + 65 * (2 * pr):][:, :65],
        rhs=e[:, 0:512], start=(kc == 0), stop=(kc == KC - 1))
    nc.tensor.matmul(
        av1[:, :], lhsT=vt_all[:, kc * LVW + 65 * (2 * pr + 1):][:, :65],
        rhs=e[:, 512:1024], start=(kc == 0), stop=(kc == KC - 1))


_NC = None


def _get_nc():
    global _NC
    if _NC is None:
        _NC = build_nc()
    return _NC


def _bf16(a):
    return np.ascontiguousarray(a.astype(ml_dtypes.bfloat16))


def make_in_maps(x, rotary_emb, w_qkv, w_out):
    x = np.asarray(x, np.float32)
    rotary_emb = np.asarray(rotary_emb, np.float32)
    w_qkv = np.asarray(w_qkv, np.float32)
    w_out = np.asarray(w_out, np.float32)
    cosT = np.cos(rotary_emb).T.astype(np.float32)  # [64, N]
    sinT = np.sin(rotary_emb).T.astype(np.float32)
    cos2_a = np.ascontiguousarray(np.concatenate([cosT, cosT], axis=0))
    sswp = np.concatenate([sinT[32:], -sinT[:32]], axis=0)
    sinm_a = np.ascontiguousarray(np.concatenate([sswp, sswp], axis=0))
    wout_bf = _bf16(w_out)
    in_maps = []
    for c in range(NC_):
        b, hb = c // 4, c % 4
        h0 = LH * hb
        wq_loc = w_qkv[:, 64 * h0 : 64 * h0 + LQK]
        wk_loc = w_qkv[:, H + 64 * h0 : H + 64 * h0 + LQK]
        wv_loc = w_qkv[:, 2 * H + 64 * h0 : 2 * H + 64 * h0 + LQK]
        wv_aug = np.zeros((H, LVW), np.float32)
        for j in range(LH):
            wv_aug[:, 65 * j : 65 * j + 64] = wv_loc[:, 64 * j : 64 * j + 64]
        msk_a = np.zeros((128, 2), np.float32)
        msk_a[:, b] = 1.0
        in_maps.append({
            "xT": _bf16(x[b].T),
            "msk": msk_a,
            "wqk": _bf16(np.concatenate([wq_loc, wk_loc], axis=1)),
            "wv": _bf16(wv_aug),
            "wout": wout_bf,
            "cos2": cos2_a,
            "sinm": sinm_a,
        })
    return in_maps


def run(x, rotary_emb, w_qkv, w_out, trace=False, tmpdir=None):
    nc = _get_nc()
    in_maps = make_in_maps(x, rotary_emb, w_qkv, w_out)
    res = run_bass_kernel_spmd(nc, in_maps, list(range(NC_)), trace=trace,
                               tmpdir=tmpdir)
    full = np.empty((B, N, H), np.float32)
    for c in range(NC_):
        b, r = c // 4, c % 4
        piece = np.asarray(res.results[c]["out"], np.float32)  # [QC, 128, H]
        for qc in range(QC):
            full[b, 512 * qc + 128 * r : 512 * qc + 128 * r + 128] = piece[qc]
    return full, res


def kernel(x, rotary_emb, w_qkv, w_out):
    full, _ = run(x, rotary_emb, w_qkv, w_out)
    return full
